# revision 32
# baseline (speedup 1.0000x reference)
"""DiffGraphTransformer attention kernel for 8x Trainium2 NeuronCores.

Reference computation (T=1024, B=8, E=512, H=8, hd=64):
    qkv = query @ in_proj_weight.T + in_proj_bias ; q,k,v = split(qkv)
    k = q ; q *= hd**-0.5
    per (batch,head): scores = q @ k.T            (T,T)
                      w = exp(scores - max) * pe[b]
                      w /= clip(sum(w,-1), 1e-6)
                      attn = w @ v
    out = attn @ out_proj_weight.T + out_proj_bias
Sharding: batch b -> core b (pure SPMD, no collectives).

Key structure (v2):
  * k == q, so only Wq / Wv of in_proj are used; softmax max-subtraction
    replaced by constant shift exp(s/8 - 10) (cancels in normalization).
  * S = q q^T symmetric; E = exp(S) stored [s, t]; W' = E * peT is the
    contraction-major attention operand (no (T,T) transpose ever).
  * attention lhsT = [v_h | ones]: PSUM row 64 = softmax denominator free.
  * ALL matmul operands fp16 (fp16 mantissa == fp32r's e8m11, but fp16
    allows standalone LDWEIGHTS that overlaps the MM stream on the 2nd
    SBUF read port; fp32/fp32r self-load weights and serialize ~110ns/MM).
  * normalization per PSUM region: reciprocal reads the denominator row
    straight from PSUM (no row-gather copies), bounces through DRAM and
    DMA-broadcasts down 64 partitions; norm multiply is fp16 2x-mode.
  * out-projection accumulates k=0..3 + bias IN PSUM (no partial
    evacuation + re-add): mt 0-3 live in the freed scores slots, mt 4-7
    in the attention slots as they evacuate.  Tail evac/copies ride the
    Scalar engine (idle after the last exp).
  * GPSIMD multiplies the FIRST 2 pe-tiles of each head (consumed one
    pipeline stage later -> its 2.7us/tile latency never blocks the PE).
  * inputs stream on both HWDGE queues (sync: wq,xT,bq; scalar: rest) so
    the first projection matmul starts ~1.5us in.
P2 is software-pipelined over head pairs: iteration j computes pair j's
scores/exp/W' while the attention matmuls consume pair j-1, with SKEW
attention k-steps hoisted above the scores at each boundary.
"""

import sys

for _p in ("/opt/trn_rl_repo",):
    if _p not in sys.path:
        sys.path.insert(0, _p)

import numpy as np

T, B, E = 1024, 8, 512
H = 8
HD = E // H  # 64
N_CORES = 8

GPSIMD_TILES = (4, 5)  # which s-tiles of each head's W' multiply go to GPSIMD
                       # (mid-iteration tiles: produced early enough, consumed
                       # mid-next-iteration, so GPSIMD's 2.2us/tile latency
                       # never blocks the boundary-hoisted attention steps)
HEATER_MMS = 2  # dummy N=512 matmuls per (pair, i) into the scores PSUM
                # (overwritten by the real start=True matmul).  They raise
                # steady-state PE busy above the HAM clock-gate threshold so
                # the PE stays at 2.4 GHz instead of oscillating with 1.2 GHz
                # stretches; they cost nothing while ACT paces the pipeline.

# global constant subtracted inside exp (cancels in normalization; keeps
# exp() outputs inside fp16 range: scores/8 - 10 is in [-16, ~6])
EXP_SHIFT = -10.0

_cache = {}


def _build_nc():
    import concourse.bass as bass
    import concourse.tile as tile
    import concourse.mybir as mybir
    from concourse import bacc
    from contextlib import ExitStack

    f32 = mybir.dt.float32
    f16 = mybir.dt.float16
    Exp = mybir.ActivationFunctionType.Exp

    nc = bacc.Bacc("TRN2", debug=False)

    # DRAM I/O (per-core contents supplied via in_maps)
    xT_d = nc.dram_tensor("xT", [E, T], f16, kind="ExternalInput").ap()
    peT_d = nc.dram_tensor("peT", [T, T], f16, kind="ExternalInput").ap()
    wqT_d = nc.dram_tensor("wqT", [E, E], f16, kind="ExternalInput").ap()
    wvT_d = nc.dram_tensor("wvT", [E, E], f16, kind="ExternalInput").ap()
    woT_d = nc.dram_tensor("woT", [E, E], f16, kind="ExternalInput").ap()
    bq_d = nc.dram_tensor("bq", [E], f32, kind="ExternalInput").ap()
    bo2_d = nc.dram_tensor("bo2", [E], f16, kind="ExternalInput").ap()
    ones_d = nc.dram_tensor("ones1", [128], f16, kind="ExternalInput").ap()
    out_d = nc.dram_tensor("out", [T, E], f32, kind="ExternalOutput").ap()

    KT = E // 128   # 4 contraction tiles for the projections
    TT = T // 128   # 8 t-tiles
    NH = T // 512   # 2 psum-bank halves of the t dimension

    with ExitStack() as ctx:
        tc = ctx.enter_context(tile.TileContext(nc))

        sing = ctx.enter_context(tc.tile_pool(name="sing", bufs=1))
        p_in = ctx.enter_context(tc.tile_pool(name="p_in", bufs=1))
        p_qv = ctx.enter_context(tc.tile_pool(name="p_qv", bufs=1))
        p_E = ctx.enter_context(tc.tile_pool(name="p_E", bufs=16))
        p_W = ctx.enter_context(tc.tile_pool(name="p_W", bufs=24))
        p_rc = ctx.enter_context(tc.tile_pool(name="p_rc", bufs=6))
        p_rm = ctx.enter_context(tc.tile_pool(name="p_rm", bufs=6))
        p_st = ctx.enter_context(tc.tile_pool(name="p_st", bufs=2))
        p_dr = ctx.enter_context(tc.tile_pool(name="p_dr", bufs=6, space="DRAM"))
        ps_a = ctx.enter_context(tc.tile_pool(name="ps_a", bufs=2, space="PSUM"))
        ps_b = ctx.enter_context(tc.tile_pool(name="ps_b", bufs=4, space="PSUM"))

        # ---- constants / weights into SBUF --------------------------------
        # Two HWDGE queues: sync carries the q-projection critical path
        # (wq[k], xT[k] alternating so the k=0 matmul starts ~1.5us in);
        # scalar carries everything else concurrently.
        wq_sb = [sing.tile([128, E], f16, tag=f"wq{k}", name="wq") for k in range(KT)]
        wv_sb = [sing.tile([128, E], f16, tag=f"wv{k}", name="wv") for k in range(KT)]
        wo_sb = [sing.tile([128, E], f16, tag=f"wo{k}", name="wo") for k in range(KT)]
        bq_sb = [sing.tile([128, 1], f32, tag=f"bq{k}", name="bq") for k in range(KT)]
        xT_sb = [p_in.tile([128, T], f16, tag=f"xT{k}", name="xT") for k in range(KT)]
        peT_sb = [p_in.tile([128, T], f16, tag=f"peT{i}", name="peT") for i in range(TT)]
        # sync queue: the qproj(0, nh=0) critical path first (wq[k] + the
        # low column half of xT[k], interleaved), then the rest.
        for k in range(KT):
            nc.sync.dma_start(out=wq_sb[k], in_=wqT_d[k * 128:(k + 1) * 128, :])
            nc.sync.dma_start(out=xT_sb[k][:, 0:512],
                              in_=xT_d[k * 128:(k + 1) * 128, 0:512])
        nc.sync.dma_start(out=bq_sb[0], in_=bq_d[0:128].rearrange("(p one) -> p one", one=1))
        for k in range(1, KT):
            nc.sync.dma_start(out=bq_sb[k], in_=bq_d[k * 128:(k + 1) * 128].rearrange("(p one) -> p one", one=1))
        ebias = sing.tile([128, 1], f32, tag="ebias")
        nc.vector.memset(ebias, EXP_SHIFT)
        zrow = sing.tile([1, 512], f16, tag="zrow")
        nc.vector.memset(zrow, 0.0)
        zcol = sing.tile([1, 128], f16, tag="zcol")
        nc.vector.memset(zcol, 0.0)
        for i in range(TT):
            nc.sync.dma_start(out=peT_sb[i], in_=peT_d[i * 128:(i + 1) * 128, :])
        for k in range(KT):
            nc.sync.dma_start(out=wo_sb[k], in_=woT_d[k * 128:(k + 1) * 128, :])
        # scalar queue (concurrently): the xT upper halves (2nd half of the
        # qproj(0,*) critical path) and wv, so the first exp fires ~14us in.
        for k in range(KT):
            nc.scalar.dma_start(out=xT_sb[k][:, 512:1024],
                                in_=xT_d[k * 128:(k + 1) * 128, 512:1024])
        for k in range(KT):
            nc.scalar.dma_start(out=wv_sb[k], in_=wvT_d[k * 128:(k + 1) * 128, :])
        ones1 = sing.tile([1, 128], f16, tag="ones1")
        nc.scalar.dma_start(out=ones1, in_=ones_d.unsqueeze(0))
        bo2_sb = sing.tile([1, E], f16, tag="bo2")
        nc.scalar.dma_start(out=bo2_sb, in_=bo2_d.unsqueeze(0))

        # ---- P1: projections ----------------------------------------------
        # qT[e_out, t] with e_out on partitions (4 tiles); includes q-bias.
        qT_sb = [p_qv.tile([128, T], f16, tag=f"qT{k}", name="qT") for k in range(KT)]
        # v[t, e_out] natural, with a ones column appended per head:
        # layout (128, 8*65): head h occupies cols [65h, 65h+64), ones at 65h+64.
        v_sb = [p_qv.tile([128, H * (HD + 1)], f16, tag=f"v{k}", name="v") for k in range(TT)]

        def emit_qproj(m, nh):
            ps = ps_b.tile([128, 512], f32, tag="slot", name="pp")
            for k in range(KT):
                nc.tensor.matmul(
                    ps, wq_sb[k][:, m * 128:(m + 1) * 128],
                    xT_sb[k][:, nh * 512:(nh + 1) * 512],
                    start=(k == 0), stop=(k == KT - 1))
            nc.vector.tensor_scalar_add(
                qT_sb[m][:, nh * 512:(nh + 1) * 512], ps, bq_sb[m])

        def emit_vproj(mt):
            ps = ps_b.tile([128, 512], f32, tag="slot", name="pp")
            for k in range(KT):
                nc.tensor.matmul(
                    ps, xT_sb[k][:, mt * 128:(mt + 1) * 128], wv_sb[k],
                    start=(k == 0), stop=(k == KT - 1))
            v_dst = v_sb[mt].rearrange("p (h c) -> p h c", c=HD + 1)
            nc.vector.tensor_copy(
                v_dst[:, :, 0:HD],
                ps.rearrange("p (h c) -> p h c", c=HD))
            nc.vector.memset(v_dst[:, :, HD:HD + 1], 1.0)

        # Dependency-free warm-up matmuls (memset operands only): the PE
        # churns through these during the input-DMA wait, so the HAM clock
        # gate is already at 2.4 GHz when the first projection matmuls run.
        warm_ps = ps_b.tile([128, 512], f32, tag="slot", name="pp")
        for _ in range(15):
            nc.tensor.matmul(warm_ps, zcol, zrow, start=True, stop=True)

        # pair 0's qT upfront; the other 14 projection groups interleave
        # with iteration 0's scores so ACT starts exp'ing early.
        for nh in range(NH):
            emit_qproj(0, nh)
        proj_rest = [("q", m, nh) for m in range(1, KT) for nh in range(NH)]
        proj_rest += [("v", mt, None) for mt in range(TT)]

        # ---- P2: attention, software-pipelined over head pairs ------------
        attnT_sb = [p_qv.tile([128, T], f16, tag=f"attnT{k}", name="attnT") for k in range(KT)]
        NP = H // 2  # pairs
        Ws_of = {}   # pair j -> [hh][i] W' tiles

        def emit_scores(j, i):
            # nh-major emission: consecutive MMs alternate row groups
            # (head A rows 0-63, head B rows 64-127).
            qt = qT_sb[j]
            scs = [ps_a.tile([128, T], f32, tag="slot", name="sc") for _ in range(2)]
            if j >= 1:
                for h_k in range(HEATER_MMS):
                    nc.tensor.matmul(
                        scs[0][:, 0:512], qt[0:HD, 0:128], qt[0:HD, 0:512],
                        start=(h_k == 0), stop=(h_k == HEATER_MMS - 1),
                        tile_position=(0, 0))
            for nh in range(NH):
                for hh in range(2):
                    r0 = hh * HD
                    nc.tensor.matmul(
                        scs[hh][:, nh * 512:(nh + 1) * 512],
                        qt[r0:r0 + HD, i * 128:(i + 1) * 128],
                        qt[r0:r0 + HD, nh * 512:(nh + 1) * 512],
                        start=True, stop=True,
                        tile_position=(r0, 0))
            for hh in range(2):
                Et = p_E.tile([128, T], f16, tag="E", name="Et")
                nc.scalar.activation(out=Et, in_=scs[hh], func=Exp, scale=0.125, bias=ebias)
                Wt = p_W.tile([128, T], f16, tag="W", name="Wt")
                if i in GPSIMD_TILES:
                    nc.gpsimd.tensor_mul(Wt, Et, peT_sb[i])
                else:
                    nc.vector.tensor_mul(Wt, Et, peT_sb[i])
                Ws_of[j][hh][i] = Wt

        def emit_attn_kstep(j, i, at_ps):
            for hh in range(2):
                vcol = 65 * (2 * j + hh)
                for nh in range(NH):
                    nc.tensor.matmul(
                        at_ps[hh][nh],
                        v_sb[i][:, vcol:vcol + HD + 1],
                        Ws_of[j][hh][i][:, nh * 512:(nh + 1) * 512],
                        start=(i == 0), stop=(i == TT - 1))

        def emit_norm_chain(j, at_ps, nhs, last):
            # Denominator reciprocal for the given nh columns: the exact
            # `reciprocal` is an iterative ~8 cyc/elem DVE op, so cost is
            # driven by per-lane free size - DMA-pack the denominator rows
            # to (n, 64) so one reciprocal covers them at FD=64, then
            # scatter through DRAM and partition-broadcast.  All fp16
            # (5e-4 rel err, gate is 2e-2).
            nseg = 2 * len(nhs)
            rows = p_rc.tile([HD + 1, 4 * 512], f16, tag="rows", name="rows",
                             bufs=4)
            for si, (hh, nh) in enumerate([(h, n) for n in nhs for h in range(2)]):
                dn = rows[HD:HD + 1, si * 512:(si + 1) * 512]
                if last:
                    nc.scalar.copy(dn, at_ps[hh][nh][HD:HD + 1, :])
                else:
                    nc.vector.tensor_copy(dn, at_ps[hh][nh][HD:HD + 1, :])
            rg = p_rc.tile([32, 64], f16, tag="rg", name="rg", bufs=4)
            nc.sync.dma_start(
                out=rg[0:8 * len(nhs) * 2, :],
                in_=rows[HD:HD + 1, 0:nseg * 512].rearrange(
                    "one (a c) -> one a c", c=64))
            rgi = p_rc.tile([32, 64], f16, tag="rgi", name="rgi", bufs=4)
            with nc.allow_low_precision(reason="softmax denom recip fp16; 5e-4 rel err vs 2e-2 gate"):
                nc.vector.reciprocal(rgi[0:8 * len(nhs) * 2, :],
                                     rg[0:8 * len(nhs) * 2, :])
            rrow = p_dr.tile([1, 4 * 512], f16, tag="rrow", name="rrow", bufs=4)
            nc.sync.dma_start(
                out=rrow[0:1, 0:nseg * 512].rearrange("one (a c) -> one a c", c=64),
                in_=rgi[0:8 * len(nhs) * 2, :])
            rms = {}
            for si, (hh, nh) in enumerate([(h, n) for n in nhs for h in range(2)]):
                rm = p_rm.tile([128, 512], f16, tag="rm", name="rm")
                seg = rrow[0:1, si * 512:(si + 1) * 512]
                bcast = bass.AP(tensor=seg.tensor, offset=seg.offset,
                                ap=[[0, HD]] + list(seg.ap[1:]))
                nc.sync.dma_start(out=rm[hh * HD:(hh + 1) * HD, :], in_=bcast)
                rms[(hh, nh)] = rm
            return rms

        def emit_norm_muls(j, rms, nhs):
            for nh in nhs:
                for hh in range(2):
                    dst = attnT_sb[j][hh * HD:(hh + 1) * HD,
                                      nh * 512:(nh + 1) * 512]
                    nc.vector.tensor_mul(dst, dst,
                                         rms[(hh, nh)][hh * HD:(hh + 1) * HD, :])

        def emit_evac(j, at_ps, last=False):
            # Per PSUM region (hh, nh): evacuate rows 0..63 to attnT (fp16)
            # and normalize.  For the tail pair the copies ride the Scalar
            # engine (idle then) and the normalization runs as TWO per-nh
            # chains so the k=3 out-projection can start after the nh=0
            # chain alone.
            if last:
                for nh in range(NH):
                    rms = emit_norm_chain(j, at_ps, [nh], last=True)
                    for hh in range(2):
                        nc.scalar.copy(
                            attnT_sb[j][hh * HD:(hh + 1) * HD,
                                        nh * 512:(nh + 1) * 512],
                            at_ps[hh][nh][0:HD, :])
                    emit_norm_muls(j, rms, [nh])
            else:
                for nh in range(NH):
                    for hh in range(2):
                        nc.vector.tensor_copy(
                            attnT_sb[j][hh * HD:(hh + 1) * HD,
                                        nh * 512:(nh + 1) * 512],
                            at_ps[hh][nh][0:HD, :])
                rms = emit_norm_chain(j, at_ps, [0, 1], last=False)
                emit_norm_muls(j, rms, [0, 1])

        # Skewed schedule: at each iteration boundary the first SKEW attention
        # k-steps (which need no fresh ACT results) run before the first
        # scores matmul, absorbing the exp-lag.
        SKEW = 3
        for j in range(NP + 1):
            if j < NP:
                Ws_of[j] = [[None] * TT, [None] * TT]
                if j >= 2:
                    del Ws_of[j - 2]
            cur_at = None
            if j >= 1:
                cur_at = [[ps_b.tile([HD + 1, 512], f32, tag="slot", name="at")
                           for _ in range(NH)] for _ in range(2)]
                for i in range(SKEW):
                    emit_attn_kstep(j - 1, i, cur_at)
            if j == NP:
                # final iteration: finish the attention, then the whole
                # out-projection accumulates IN PSUM: mt 0-3 in the freed
                # scores slots (halves of the (128,1024) slots), mt 4-7 in
                # the attention slots as they evacuate.  k=3 (pair 3's
                # rows) joins after the pair-3 normalization, then bias,
                # one copy to SBUF, store.
                for i in range(SKEW, TT):
                    emit_attn_kstep(j - 1, i, cur_at)

                p3ps = []
                big = [ps_a.tile([128, T], f32, tag="slot", name="sc")
                       for _ in range(2)]
                for mt in range(4):
                    p3ps.append(big[mt // 2][:, (mt % 2) * 512:(mt % 2 + 1) * 512])

                def emit_p3_partial(mt):
                    for k in range(KT - 1):
                        nc.tensor.matmul(
                            p3ps[mt], attnT_sb[k][:, mt * 128:(mt + 1) * 128],
                            wo_sb[k], start=(k == 0), stop=False)

                for mt in range(4):
                    emit_p3_partial(mt)
                emit_evac(j - 1, cur_at, last=True)
                # zero-adding filler matmuls (ones1.T @ zeros == 0) bridge the
                # evacuation + normalization-chain latency so the HAM clock
                # gate never re-throttles before the final k=3 matmuls.
                for _ in range(4):
                    nc.tensor.matmul(p3ps[0], ones1, zrow, start=False, stop=False)
                for mt in range(4, TT):
                    p3ps.append(ps_b.tile([128, 512], f32, tag="slot", name="pp"))
                    emit_p3_partial(mt)
                for _ in range(8):
                    nc.tensor.matmul(p3ps[1], ones1, zrow, start=False, stop=False)
                for mt in range(TT):
                    nc.tensor.matmul(
                        p3ps[mt], attnT_sb[KT - 1][:, mt * 128:(mt + 1) * 128],
                        wo_sb[KT - 1], start=False, stop=False)
                    nc.tensor.matmul(p3ps[mt], ones1, bo2_sb, start=False, stop=True)
                    # evacuate + store immediately so stores overlap the
                    # remaining matmuls; alternate engines and DMA queues.
                    st = p_st.tile([128, E], f32, tag="st", name="st", bufs=4)
                    if mt % 2 == 0:
                        nc.scalar.copy(st, p3ps[mt])
                    else:
                        nc.vector.tensor_copy(st, p3ps[mt])
                    q = nc.sync if mt % 2 == 0 else nc.scalar
                    q.dma_start(out=out_d[mt * 128:(mt + 1) * 128, :], in_=st)
            else:
                for i in range(TT):
                    emit_scores(j, i)
                    if j == 0 and i >= 2:
                        # defer the projection interleave 2 tiles so the
                        # first exps stream without competing PE work
                        for _ in range(3):
                            if proj_rest:
                                kind, a1, a2 = proj_rest.pop(0)
                                if kind == "q":
                                    emit_qproj(a1, a2)
                                else:
                                    emit_vproj(a1)
                    if j >= 1 and i + SKEW < TT:
                        emit_attn_kstep(j - 1, i + SKEW, cur_at)
                    if j >= 1 and i == TT - SKEW - 1:
                        # evacuate pair j-1 right after its LAST attention
                        # k-step is emitted (not at iteration end): its at_ps
                        # slots recycle ~6us earlier, so the next boundary's
                        # hoisted attention never stalls on slot reuse.  The
                        # W' muls queued behind have a full iteration of
                        # slack before consumption.
                        emit_evac(j - 1, cur_at)

    nc.compile()
    return nc


def get_nc():
    if "nc" not in _cache:
        _cache["nc"] = _build_nc()
    return _cache["nc"]


def prep_inputs(query, pe, in_proj_weight, in_proj_bias, out_proj_weight,
                out_proj_bias):
    """Host-side sharding/layout prep. Returns per-core input maps."""
    query = np.asarray(query, dtype=np.float32)
    pe = np.asarray(pe, dtype=np.float32)
    in_proj_weight = np.asarray(in_proj_weight, dtype=np.float32)
    in_proj_bias = np.asarray(in_proj_bias, dtype=np.float32)
    out_proj_weight = np.asarray(out_proj_weight, dtype=np.float32)
    out_proj_bias = np.asarray(out_proj_bias, dtype=np.float32)

    wqT = in_proj_weight[0:E].T.astype(np.float16)             # (E, E)
    wvT = in_proj_weight[2 * E:3 * E].T.astype(np.float16)     # (E, E)
    woT = out_proj_weight.T.astype(np.float16)                 # (E, E)
    bq = np.ascontiguousarray(in_proj_bias[0:E])
    bv = in_proj_bias[2 * E:3 * E]
    bo2 = (out_proj_weight @ bv + out_proj_bias).astype(np.float16)

    in_maps = []
    for b in range(N_CORES):
        xT = query[:, b, :].T.astype(np.float16)               # (E, T)
        peT = np.ascontiguousarray(pe[b].T).astype(np.float16)
        in_maps.append({
            "xT": xT, "peT": peT, "wqT": wqT, "wvT": wvT, "woT": woT,
            "bq": bq, "bo2": bo2, "ones1": np.ones(128, dtype=np.float16),
        })
    return in_maps


def kernel(query, pe, in_proj_weight, in_proj_bias, out_proj_weight,
           out_proj_bias):
    from concourse.bass_utils import run_bass_kernel_spmd

    nc = get_nc()
    in_maps = prep_inputs(query, pe, in_proj_weight, in_proj_bias,
                          out_proj_weight, out_proj_bias)
    res = run_bass_kernel_spmd(nc, in_maps, list(range(N_CORES)))
    out = np.empty((T, B, E), dtype=np.float32)
    for b in range(N_CORES):
        out[:, b, :] = res.results[b]["out"]
    return out
